# revision 1
# baseline (speedup 1.0000x reference)
"""EdgeGCN Trainium2 kernel: 2-layer GCN + all-pairs affinity + triu sigmoid.

Self-contained: hardcodes the problem shapes (N=10000, E=320000, F=128, H=16)
and the 8-core sharding.

Strategy (per core c, SPMD-uniform program):
  - Pad N -> NPAD=10240 = 8 shards x 1280 nodes; core c owns dst nodes
    [1280c, 1280(c+1)). Host appends self-loop edges (n,n) for every node,
    buckets edges by dst block of 128, and lays each block's edges into
    [128, CPB] slots split into four fixed-size q-runs by q = src mod 4.
  - Layer 1 aggregates in x-space with a HOST-gathered x[src] slot tensor
    (bulk-DMA input, no device gather):
      per chunk: oh' = (iota == dst_local) * dinv[src]   (one DVE op)
                 psum[128F,128d] += xg_chunk^T-contraction oh'   (one matmul)
      per block: S^T -> @W1 -> transpose -> dinv-scale, +b1, relu -> h1,
                 v = dinv * h1.
  - AllGather v -> DRAM; layer 2 gathers v[src] via dma_gather over a
    [NPAD/4, 64] view (4 nodes per 256B row; the q-runs make the 16-col
    sub-slice offset compile-time), one-hot matmuls accumulate S^T[16,128d],
    then @W2, transpose, dinv-scale, +b2 -> h2. AllGather h2.
  - Affinity: core c owns row blocks b = 8i + c; block i streams
    h2T[:, 1024i:NPAD] through the PE in 512-col chunks (contraction=16),
    sigmoid on ACT, two rectangular DMAs per block to a padded output
    region. The host slices out the exact packed triu segments.
"""

import numpy as np

NCORES = 8
F = 128
H = 16


def _cfg(N, NPAD, CPB):
    SH = NPAD // NCORES
    assert SH % 128 == 0 and CPB % 4 == 0
    BPC = SH // 128
    return dict(N=N, NPAD=NPAD, CPB=CPB, SH=SH, BPC=BPC, NBLK=NPAD // 128)


FULL = _cfg(N=10000, NPAD=10240, CPB=40)


# ---------------------------------------------------------------- device ----

def build_nc(cfg, debug=False):
    import concourse.bass as bass
    import concourse.mybir as mybir
    import concourse.tile as tile
    from concourse import bacc

    NPAD, SH, BPC, CPB, NBLK = (cfg[k] for k in ("NPAD", "SH", "BPC", "CPB", "NBLK"))
    CPQ = CPB // 4            # chunks per q-run (q = src mod 4)
    G = BPC * CPB             # total chunks
    NIDX = CPB * 64           # gather indices per HALF dst block (2 q-runs)
    HC = CPB // 2             # chunks per half block
    UW = 64                   # v table viewed as [NPAD/4, 64]: 4 nodes / 256B
    AW = 5120                 # affinity output column strip width
    f32 = mybir.dt.float32
    i32 = mybir.dt.int32
    i16 = mybir.dt.int16
    AF = mybir.ActivationFunctionType
    OP = mybir.AluOpType
    RG = [list(range(NCORES))]

    nc = bacc.Bacc("TRN2", target_bir_lowering=False, debug=False,
                   enable_asserts=True, num_devices=NCORES,
                   num_swdge_queues=4)

    W1 = nc.dram_tensor("W1", [F, H], f32, kind="ExternalInput").ap()
    W2 = nc.dram_tensor("W2", [2 * H, H], f32, kind="ExternalInput").ap()
    b1 = nc.dram_tensor("b1", [128, H], f32, kind="ExternalInput").ap()
    b2 = nc.dram_tensor("b2", [128, H], f32, kind="ExternalInput").ap()
    deg = nc.dram_tensor("deg", [128, BPC], f32, kind="ExternalInput").ap()
    dgs = nc.dram_tensor("dgs", [128, G], f32, kind="ExternalInput").ap()
    xg = nc.dram_tensor("xg", [128, G, F], f32, kind="ExternalInput").ap()
    gidx = nc.dram_tensor("gidx", [128, 2 * BPC * (NIDX // 16)], i16,
                          kind="ExternalInput").ap()
    dloc1 = nc.dram_tensor("dloc1", [128, G], f32, kind="ExternalInput").ap()
    dloc = nc.dram_tensor("dloc", [128, G], f32, kind="ExternalInput").ap()
    iota = nc.dram_tensor("iota", [128, 512], f32, kind="ExternalInput").ap()
    ident = nc.dram_tensor("ident", [128, 128], f32, kind="ExternalInput").ap()
    rowi = nc.dram_tensor("rowi", [128, BPC], i32, kind="ExternalInput").ap()
    outs = [nc.dram_tensor(f"out{i}", [128, NPAD - 1024 * i], f32,
                           kind="ExternalOutput").ap() for i in range(BPC)]

    bf16 = mybir.dt.bfloat16
    vb = nc.dram_tensor("vb", [SH, 2 * H], bf16)
    vf = nc.dram_tensor("vf", [NPAD, 2 * H], bf16, addr_space="Shared")
    hb = nc.dram_tensor("hb", [SH, H], f32)
    h2f = nc.dram_tensor("h2f", [NPAD, H], f32, addr_space="Shared")

    def nmaj_dram(t):  # [SH, H] viewed as [128, BPC, H]
        return t.ap().rearrange("(j p) f -> p j f", p=128)

    with tile.TileContext(nc) as tc:
        from contextlib import ExitStack as _ES
        with _ES() as _stk:
            cp = _stk.enter_context(tc.tile_pool(name="const", bufs=1))
            wp = _stk.enter_context(tc.tile_pool(name="work", bufs=3))
            ohp = _stk.enter_context(tc.tile_pool(name="ohp", bufs=8))
            psC = _stk.enter_context(tc.tile_pool(name="psC", bufs=2, space="PSUM"))
            _agg = _ES()
            xgp = _agg.enter_context(tc.tile_pool(name="xgp", bufs=4))
            gp = _agg.enter_context(tc.tile_pool(name="gp", bufs=8))
            psA = _agg.enter_context(tc.tile_pool(name="psA", bufs=2, space="PSUM"))
            psB = _agg.enter_context(tc.tile_pool(name="psB", bufs=2, space="PSUM"))

            def load(name, ap_in, shape, dtype=f32, pool=cp):
                t = pool.tile(shape, dtype, tag=name)
                nc.sync.dma_start(out=t[:], in_=ap_in)
                return t

            with nc.named_scope("load"):
                W1_t = load("W1", W1, [F, H])
                W2_t = load("W2", W2, [2 * H, H])
                b1_t = load("b1", b1, [128, H])
                b2_t = load("b2", b2, [128, H])
                deg_t = load("deg", deg, [128, BPC])
                dgs_t = load("dgs", dgs, [128, G])
                gidx_t = load("gidx", gidx, [128, 2 * BPC * (NIDX // 16)], i16)
                dloc1_t = load("dloc1", dloc1, [128, G])
                dloc_t = load("dloc", dloc, [128, G])
                iota_t = load("iota", iota, [128, 512])
                ident_t = load("ident", ident, [128, 128])
                rowi_t = load("rowi", rowi, [128, BPC], i32)
                dinv_t = cp.tile([128, BPC], f32)
                rec = wp.tile([128, BPC], f32, tag="rec")
                nc.vector.reciprocal(rec[:], deg_t[:])
                nc.scalar.activation(dinv_t[:], rec[:], AF.Sqrt)
                dis_t = cp.tile([128, G], f32)
                rec2 = wp.tile([128, G], f32, tag="rec2")
                nc.vector.reciprocal(rec2[:], dgs_t[:])
                nc.scalar.activation(dis_t[:], rec2[:], AF.Sqrt)

            v_t = cp.tile([128, BPC * H], f32)
            vhl_t = cp.tile([128, BPC * 2 * H], bf16)
            h2_t = cp.tile([128, BPC * H], f32)

            # ---------------- layer 1: x-space aggregation ----------------
            with nc.named_scope("l1agg"):
                for j in range(BPC):
                    pre = psA.tile([F, 128], f32, tag="agg")
                    for hh in range(2):
                        xgt = xgp.tile([128, HC * F], f32, tag="xg")
                        t0 = CPB * j + hh * HC
                        nc.sync.dma_start(
                            out=xgt[:].rearrange("p (c k) -> p c k", k=F),
                            in_=xg[:, t0:t0 + HC, :])
                        for tl in range(HC):
                            t = hh * HC + tl
                            c0 = CPB * j + t
                            oh = ohp.tile([128, 128], f32, tag="oh1")
                            nc.vector.tensor_scalar(
                                oh[:], iota_t[:, 0:128], dloc1_t[:, c0:c0 + 1],
                                dis_t[:, c0:c0 + 1],
                                op0=OP.is_equal, op1=OP.mult)
                            nc.tensor.matmul(
                                pre[:], lhsT=xgt[:, tl * F:(tl + 1) * F],
                                rhs=oh[:],
                                start=(t == 0), stop=(t == CPB - 1))
                    preS = wp.tile([F, 128], f32, tag="preS")
                    nc.vector.tensor_copy(preS[:], pre[:])
                    pw = psB.tile([128, H], f32, tag="dens")
                    nc.tensor.matmul(pw[:], lhsT=preS[:], rhs=W1_t[:],
                                     start=True, stop=True)
                    s = wp.tile([128, H], f32, tag="ep")
                    nc.vector.tensor_scalar_mul(s[:], pw[:], dinv_t[:, j:j + 1])
                    nc.vector.tensor_add(s[:], s[:], b1_t[:])
                    nc.vector.tensor_scalar_max(s[:], s[:], 0.0)
                    vj = v_t[:, H * j:H * (j + 1)]
                    nc.vector.tensor_scalar_mul(vj, s[:], dinv_t[:, j:j + 1])
                    hi = vhl_t[:, 2 * H * j:2 * H * j + H]
                    lof = wp.tile([128, H], f32, tag="lof")
                    nc.vector.tensor_copy(hi, vj)
                    nc.vector.tensor_copy(lof[:], hi)
                    nc.vector.tensor_tensor(
                        out=vhl_t[:, 2 * H * j + H:2 * H * (j + 1)],
                        in0=vj, in1=lof[:], op=OP.subtract)
                nc.sync.dma_start(
                    out=vb.ap().rearrange("(j p) f -> p j f", p=128),
                    in_=vhl_t[:].rearrange("p (j f) -> p j f", f=2 * H))
            nc.gpsimd.collective_compute("AllGather", OP.bypass, replica_groups=RG,
                                         ins=[vb.ap().opt()], outs=[vf.ap().opt()])

            # ---------------- layer 2: dma_gather aggregation -------------
            v4 = vf.ap().rearrange("(m q) f -> m (q f)", q=4)
            UWB = 128  # bf16 elements per gather row
            with nc.named_scope("l2agg"):
                NSUB = 4                      # sub-gathers per half block
                SC = HC // NSUB               # chunks per sub-gather
                for j in range(BPC):
                    gs = []
                    for hh in range(2):
                        g = gp.tile([128, HC * UWB], bf16, tag="gath")
                        i0 = (2 * j + hh) * (NIDX // 16)
                        for sb in range(NSUB):
                            w = (NIDX // NSUB) // 16
                            nc.gpsimd.dma_gather(
                                out_ap=g[:, sb * SC * UWB:(sb + 1) * SC * UWB]
                                    .rearrange("p (c e) -> p c e", e=UWB),
                                in_ap=v4,
                                idxs_ap=gidx_t[:, i0 + sb * w:i0 + (sb + 1) * w],
                                num_idxs=NIDX // NSUB,
                                num_idxs_reg=NIDX // NSUB,
                                elem_size=UWB,
                                single_packet=False,
                                queue_num=((2 * j + hh) * NSUB + sb) % 4,
                            )
                        gs.append(g)
                    psT = psA.tile([2 * H, 128], f32, tag="agg")
                    for t in range(CPB):
                        q = t // CPQ
                        if t % 4 == 0:
                            oh = ohp.tile([128, 512], bf16, tag="oh2")
                            c0 = CPB * j + t
                            nc.vector.tensor_tensor(
                                out=oh[:].rearrange("p (c d) -> p c d", d=128),
                                in0=iota_t[:].rearrange("p (c d) -> p c d", d=128),
                                in1=dloc_t[:, c0:c0 + 4, None].to_broadcast(
                                    [128, 4, 128]),
                                op=OP.is_equal)
                        g = gs[t // HC]
                        tl = t % HC
                        nc.tensor.matmul(
                            psT[:],
                            lhsT=g[:, tl * UWB + 2 * H * q: tl * UWB + 2 * H * (q + 1)],
                            rhs=oh[:, (t % 4) * 128:(t % 4) * 128 + 128],
                            start=(t == 0), stop=(t == CPB - 1))
                    ST = wp.tile([2 * H, 128], f32, tag="ST")
                    nc.vector.tensor_copy(ST[:], psT[:])
                    pT = psC.tile([H, 128], f32, tag="tps")
                    nc.tensor.matmul(pT[:], lhsT=W2_t[:], rhs=ST[:],
                                     start=True, stop=True)
                    wT = wp.tile([H, 128], f32, tag="wT")
                    nc.vector.tensor_copy(wT[:], pT[:])
                    pw = psB.tile([128, H], f32, tag="dens")
                    nc.tensor.transpose(pw[:], wT[:], ident_t[0:H, 0:H])
                    s = wp.tile([128, H], f32, tag="ep")
                    nc.vector.tensor_scalar_mul(s[:], pw[:], dinv_t[:, j:j + 1])
                    nc.vector.tensor_add(h2_t[:, H * j:H * (j + 1)], s[:], b2_t[:])
                with nc.named_scope("hgather"):
                    nc.sync.dma_start(
                        out=nmaj_dram(hb),
                        in_=h2_t[:].rearrange("p (j f) -> p j f", f=H))
            nc.gpsimd.collective_compute("AllGather", OP.bypass, replica_groups=RG,
                                         ins=[hb.ap().opt()], outs=[h2f.ap().opt()])
            _agg.close()
            psE = _stk.enter_context(tc.tile_pool(name="psE", bufs=4, space="PSUM"))
            widep = _stk.enter_context(tc.tile_pool(name="widep", bufs=4))

            # ---------------- affinity + sigmoid + packed writes ----------
            with nc.named_scope("affprep"):
                h2n_t = cp.tile([128, NBLK * H], f32)
                nc.sync.dma_start(
                    out=h2n_t[:].rearrange("p (k f) -> p k f", f=H),
                    in_=h2f.ap().rearrange("(k p) f -> p k f", p=128))
                h2T_t = cp.tile([H, NPAD], f32)
                for k in range(NBLK):
                    pt = psC.tile([H, 128], f32, tag="tps")
                    nc.tensor.transpose(pt[:], h2n_t[:, H * k:H * (k + 1)],
                                        ident_t[:])
                    nc.vector.tensor_copy(h2T_t[:, 128 * k:128 * (k + 1)], pt[:])
                lhsTs = []
                for i in range(BPC):
                    hr = wp.tile([128, H], f32, tag="hr")
                    nc.gpsimd.indirect_dma_start(
                        out=hr[:], out_offset=None, in_=h2f.ap(),
                        in_offset=bass.IndirectOffsetOnAxis(
                            ap=rowi_t[:, i:i + 1], axis=0))
                    pt = psC.tile([H, 128], f32, tag="tps")
                    nc.tensor.transpose(pt[:], hr[:], ident_t[:])
                    lt = cp.tile([H, 128], f32, tag=f"lhsT{i}")
                    nc.vector.tensor_copy(lt[:], pt[:])
                    lhsTs.append(lt)

            with nc.named_scope("aff"):
                for i in range(BPC):
                    Wi = NPAD - 1024 * i
                    for a0 in range(0, Wi, AW):
                        aw = min(AW, Wi - a0)
                        wt = widep.tile([128, AW], f32, tag="wide")
                        for k in range(aw // 512):
                            pa = psE.tile([128, 512], f32, tag="affps")
                            c0 = 1024 * i + a0 + 512 * k
                            nc.tensor.matmul(pa[:], lhsT=lhsTs[i][:],
                                             rhs=h2T_t[:, c0:c0 + 512],
                                             start=True, stop=True)
                            nc.scalar.activation(wt[:, 512 * k:512 * (k + 1)],
                                                 pa[:], AF.Sigmoid)
                        nc.sync.dma_start(out=outs[i][:, a0:a0 + aw],
                                          in_=wt[:, 0:aw])

            if debug:
                d = nc.dram_tensor("dbg_vf", [NPAD, 2 * H], bf16,
                                   kind="ExternalOutput")
                nc.sync.dma_start(out=d.ap(), in_=vf.ap())
                d = nc.dram_tensor("dbg_h2f", [NPAD, H], f32,
                                   kind="ExternalOutput")
                nc.sync.dma_start(out=d.ap(), in_=h2f.ap())

    nc.compile()
    return nc


# ------------------------------------------------------------------ host ----

def preprocess(x, edge_index, W1, b1, W2, b2, cfg):
    """Build the 8 per-core input maps. Returns (in_maps, cpb_needed)."""
    N, NPAD, SH, BPC, CPB = (cfg[k] for k in ("N", "NPAD", "SH", "BPC", "CPB"))
    G = BPC * CPB
    CPQ = CPB // 4
    NIDX = CPB * 64
    HC = CPB // 2

    x = np.asarray(x, dtype=np.float32)
    src = np.asarray(edge_index[0], dtype=np.int64)
    dst = np.asarray(edge_index[1], dtype=np.int64)
    W1 = np.asarray(W1, np.float32)
    W2 = np.vstack([np.asarray(W2, np.float32)] * 2)
    b1 = np.asarray(b1, np.float32).reshape(1, H)
    b2 = np.asarray(b2, np.float32).reshape(1, H)

    xp = np.zeros((NPAD, F), np.float32)
    xp[:N] = x
    deg = (np.bincount(dst, minlength=NPAD) + 1).astype(np.float32)

    # append self loops for every (padded) node
    loop = np.arange(NPAD, dtype=np.int64)
    s_all = np.concatenate([src, loop])
    d_all = np.concatenate([dst, loop])
    order = np.argsort(d_all, kind="stable")
    s_s = s_all[order].astype(np.int32)
    d_s = d_all[order].astype(np.int32)

    iota = np.broadcast_to(np.tile(np.arange(128, dtype=np.float32), 4),
                           (128, 512)).copy()
    ident = np.eye(128, dtype=np.float32)
    b1b = np.broadcast_to(b1, (128, H)).copy()
    b2b = np.broadcast_to(b2, (128, H)).copy()

    in_maps = []
    cpb_needed = 0
    for c in range(NCORES):
        lo, hi = SH * c, SH * (c + 1)
        a, b = np.searchsorted(d_s, [lo, hi])
        s_c, d_c = s_s[a:b], d_s[a:b]
        blk = (d_c - lo) // 128
        bounds = np.searchsorted(blk, np.arange(BPC + 1))
        slot_src = np.zeros((128, G), np.int64)
        dl = np.full((128, G), -1.0, np.float32)
        slot_src1 = np.zeros((128, G), np.int64)
        dl1 = np.full((128, G), -1.0, np.float32)
        for j in range(BPC):
            sl = slice(bounds[j], bounds[j + 1])
            sj, dj = s_c[sl], d_c[sl]
            for q in range(4):
                # layer-2 layout: q-runs by src mod 4 (gather sub-slice)
                qe = np.nonzero((sj & 3) == q)[0]
                qe = qe[np.argsort(sj[qe], kind="stable")]
                m = len(qe)
                cpb_needed = max(cpb_needed, 4 * (-(-m // 128)))
                if m > CPQ * 128:
                    return None, cpb_needed
                e = np.arange(m)
                t = CPB * j + q * CPQ + e // 128
                p = e % 128
                slot_src[p, t] = sj[qe]
                dl[p, t] = dj[qe] - lo - 128 * j
                # layer-1 layout: w-runs by 32-wide dst window
                we = np.nonzero((dj - lo - 128 * j) // 32 == q)[0]
                m = len(we)
                cpb_needed = max(cpb_needed, 4 * (-(-m // 128)))
                if m > CPQ * 128:
                    return None, cpb_needed
                e = np.arange(m)
                t = CPB * j + q * CPQ + e // 128
                p = e % 128
                slot_src1[p, t] = sj[we]
                dl1[p, t] = dj[we] - lo - 128 * j
        # layer-2 gather indices: src//4 per slot, 16-wrapped per half block
        gidx = np.zeros((128, 2 * BPC * (NIDX // 16)), np.int16)
        grp = (slot_src >> 2).astype(np.int16)  # [128, G]
        NSUB, SUB = 4, NIDX // 4
        for j in range(BPC):
            for hh in range(2):
                cols = slice(CPB * j + hh * HC, CPB * j + (hh + 1) * HC)
                flat = grp[:, cols].T.reshape(-1)       # k = c*128 + p
                i0 = (2 * j + hh) * (NIDX // 16)
                for sb in range(NSUB):
                    fs = flat[sb * SUB:(sb + 1) * SUB]
                    k = np.arange(SUB)
                    w16 = np.zeros((16, SUB // 16), np.int16)
                    w16[k % 16, k // 16] = fs
                    c0 = i0 + sb * (SUB // 16)
                    gidx[:, c0:c0 + SUB // 16] = np.tile(w16, (8, 1))

        p = np.arange(128)
        jj = np.arange(BPC)
        deg_nm = deg[lo + 128 * jj[None, :] + p[:, None]]
        rowi = (128 * (8 * jj[None, :] + c) + p[:, None]).astype(np.int32)

        in_maps.append({
            "W1": W1, "W2": W2, "b1": b1b, "b2": b2b,
            "deg": np.ascontiguousarray(deg_nm, dtype=np.float32),
            "dgs": deg[slot_src1],
            "xg": xp[slot_src1],
            "gidx": gidx, "dloc": dl, "dloc1": dl1,
            "iota": iota, "ident": ident, "rowi": rowi,
        })
    return in_maps, cpb_needed


def assemble(results, cfg):
    N, NPAD, BPC = cfg["N"], cfg["NPAD"], cfg["BPC"]
    T = N * (N - 1) // 2
    row_off = np.zeros(N + 1, np.int64)
    np.cumsum((N - 1) - np.arange(N), out=row_off[1:])
    out = np.empty(T, np.float32)
    for c in range(NCORES):
        for i in range(BPC):
            reg = results[c][f"out{i}"]
            r0 = 128 * (8 * i + c)
            if r0 >= N - 1:
                continue
            base = 1024 * i
            for p in range(min(128, N - 1 - r0)):
                r = r0 + p
                L = N - 1 - r
                cs = r + 1 - base
                out[row_off[r]:row_off[r] + L] = reg[p, cs:cs + L]
    return out.reshape(-1, 1)


_NC_CACHE = {}


def _get_nc(cfg, debug=False):
    key = (cfg["NPAD"], cfg["CPB"], debug)
    if key not in _NC_CACHE:
        _NC_CACHE[key] = build_nc(cfg, debug=debug)
    return _NC_CACHE[key]


def run(inputs, cfg, trace=False, trace_kwargs=None, debug=False):
    """Run the kernel for the given cfg; returns (BassKernelResults, cfg)."""
    from concourse.bass_utils import run_bass_kernel_spmd

    in_maps, cpb_needed = preprocess(
        inputs["x"], inputs["edge_index"], inputs["W1"], inputs["b1"],
        inputs["W2"], inputs["b2"], cfg)
    if in_maps is None:
        cfg = dict(cfg, CPB=cpb_needed)
        in_maps, _ = preprocess(
            inputs["x"], inputs["edge_index"], inputs["W1"], inputs["b1"],
            inputs["W2"], inputs["b2"], cfg)
    nc = _get_nc(cfg, debug=debug)
    res = run_bass_kernel_spmd(nc, in_maps, core_ids=list(range(NCORES)),
                               trace=trace, **(trace_kwargs or {}))
    return res, cfg


def kernel(**inputs) -> np.ndarray:
    res, cfg = run(inputs, FULL, trace=False)
    return assemble(res.results, cfg)


if __name__ == "__main__":
    pass



# revision 14
# speedup vs baseline: 3.1215x; 3.1215x over previous
"""EdgeGCN Trainium2 kernel: 2-layer GCN + all-pairs affinity + triu sigmoid.

Self-contained: hardcodes the problem shapes (N=10000, E=320000, F=128, H=16)
and the 8-core sharding.

Strategy (per core c, SPMD-uniform program; all matmul inputs fp8/bf16):
  - Pad N -> NPAD=10240 = 8 shards x 1280 nodes; core c owns dst nodes
    [1280c, 1280(c+1)).
  - Layer 1 aggregates in x-space with HOST-gathered fp8 slot tensors:
    xg[p,t,:] = (x*dinv)[src] and a unit one-hot oh1[p, t*128+dloc] = 1.
    Per dst block: psum[128f,128d] += xg_chunk^T-contract oh1_chunk (fp8
    matmuls, FWL), then @W1 (bf16), transpose to node-major, exact-f32
    dinv scaling + relu -> v table [SH,16] fp8.  AllGather v.
  - Layer 2 is gather-free: host ships T2[p, w, dl] = sum(mult)*dinv[dst]
    (fp8) mapping src-window w (128 srcs) -> local dst.  The v table loads
    to SBUF as [128, 80, 16]; fp8 DoubleRow matmuls contract TWO windows
    per pass: S[16, dgrp] += sum_w u[:,2w:2w+2,:]^T-contract T2 pair.
    Then @W2 (bf16) -> h2T [16, 1280] bf16 (feature-major, no transposes).
    AllGather h2T -> h2f [128, 1280].
  - Affinity: af2[16, 10240] bf16 in SBUF; per-core row blocks k=8i+c get
    lhsT [16,128] via indirect DMA over a flat view of h2f; 512-col bf16
    matmuls -> [128, 2048] psum; sigmoid on ACT -> bf16; two rectangular
    DMAs per block row to padded bf16 outputs.  Host slices the packed
    triu segments and converts to f32.
"""

import numpy as np
import ml_dtypes

NCORES = 8
F = 128
H = 16
N = 10000
NPAD = 10240
SH = NPAD // NCORES          # 1280 nodes per shard
BPC = SH // 128              # 10 dst blocks per core
NBLK = NPAD // 128           # 80 row blocks total
NW = NPAD // 128             # 80 src windows
NWP = NW // 2                # 40 window pairs (DoubleRow)
AW = 5120                    # affinity staging strip width

F8 = ml_dtypes.float8_e4m3
BF = ml_dtypes.bfloat16


def _cfg(CPB1):
    return dict(CPB1=CPB1, G1=BPC * CPB1)


FULL = _cfg(CPB1=36)


# ---------------------------------------------------------------- device ----

def build_nc(cfg, debug=False):
    import concourse.bass as bass
    import concourse.mybir as mybir
    import concourse.tile as tile
    from concourse import bacc

    CPB1, G1 = cfg["CPB1"], cfg["G1"]
    HC = (CPB1 + 1) // 2          # chunks per half-block load
    f32 = mybir.dt.float32
    i32 = mybir.dt.int32
    bf16 = mybir.dt.bfloat16
    f8 = mybir.dt.float8e4
    AF = mybir.ActivationFunctionType
    OP = mybir.AluOpType
    PAIRS_PER_TILE = 8            # T2 streamed in tiles of 8 window pairs
    NT2 = NWP // PAIRS_PER_TILE   # 5 T2 tiles
    RG = [list(range(NCORES))]

    nc = bacc.Bacc("TRN2", target_bir_lowering=False, debug=False,
                   enable_asserts=True, num_devices=NCORES,
                   num_swdge_queues=4)

    W1 = nc.dram_tensor("W1", [F, H], bf16, kind="ExternalInput").ap()
    W2 = nc.dram_tensor("W2", [H, H], bf16, kind="ExternalInput").ap()
    b1 = nc.dram_tensor("b1", [128, H], f32, kind="ExternalInput").ap()
    b2 = nc.dram_tensor("b2", [H, 1], f32, kind="ExternalInput").ap()
    dnv = nc.dram_tensor("dnv", [128, BPC], f32, kind="ExternalInput").ap()
    xg = nc.dram_tensor("xg", [128, G1, F], f8, kind="ExternalInput").ap()
    oh1 = nc.dram_tensor("oh1", [128, G1 * 128], f8, kind="ExternalInput").ap()
    T2 = nc.dram_tensor("T2", [128, NWP, 2 * SH], f8, kind="ExternalInput").ap()
    ident = nc.dram_tensor("ident", [H, H], f32, kind="ExternalInput").ap()
    rowi = nc.dram_tensor("rowi", [H, BPC], i32, kind="ExternalInput").ap()
    dumi = nc.dram_tensor("dumi", [8, 4], f32)
    outs = [nc.dram_tensor(f"out{i}", [128, NPAD - 1024 * i], bf16,
                           kind="ExternalOutput").ap() for i in range(BPC)]

    vb = nc.dram_tensor("vb", [SH, H], f8)
    vf = nc.dram_tensor("vf", [NPAD, H], f8, addr_space="Shared")
    hb = nc.dram_tensor("hb", [H, SH], bf16)
    h2f = nc.dram_tensor("h2f", [128, SH], bf16, addr_space="Shared")
    dumo = nc.dram_tensor("dumo", [64, 4], f32, addr_space="Shared")

    with tile.TileContext(nc) as tc:
        from contextlib import ExitStack as _ES
        with _ES() as _stk:
            cp = _stk.enter_context(tc.tile_pool(name="const", bufs=1))
            wp = _stk.enter_context(tc.tile_pool(name="work", bufs=3))
            _agg = _ES()
            xgp = _agg.enter_context(tc.tile_pool(name="xgp", bufs=3))
            ohp = _agg.enter_context(tc.tile_pool(name="ohp", bufs=3))
            t2p = _agg.enter_context(tc.tile_pool(name="t2p", bufs=2))
            psA = _agg.enter_context(tc.tile_pool(name="psA", bufs=2, space="PSUM"))
            psB = _agg.enter_context(tc.tile_pool(name="psB", bufs=1, space="PSUM"))
            psS = _agg.enter_context(tc.tile_pool(name="psS", bufs=1, space="PSUM"))

            def load(name, ap_in, shape, dtype=f32, pool=cp):
                t = pool.tile(shape, dtype, tag=name)
                nc.sync.dma_start(out=t[:], in_=ap_in)
                return t

            with nc.named_scope("load"):
                W1_t = load("W1", W1, [F, H], bf16)
                W2_t = load("W2", W2, [H, H], bf16)
                b1_t = load("b1", b1, [128, H])
                b2_t = load("b2", b2, [H, 1])
                dnv_t = load("dnv", dnv, [128, BPC])
                ident_t = load("ident", ident, [H, H])
                rowi_t = load("rowi", rowi, [H, BPC], i32)
                nc.sync.dma_start(out=dumi.ap(), in_=ident_t[0:8, 0:4])
            # dummy collective early: absorbs the bootstrap barrier
            nc.gpsimd.collective_compute("AllGather", OP.bypass, replica_groups=RG,
                                         ins=[dumi.ap().opt()], outs=[dumo.ap().opt()])

            vcol_t = cp.tile([128, BPC * H], f8)
            h2T_t = cp.tile([H, SH], bf16)

            # ---------------- layer 1: x-space slot aggregation -----------
            with nc.named_scope("l1agg"):
                for j in range(BPC):
                    pre = psA.tile([F, 128], f32, tag="pre")
                    for hh in range(2):
                        t0 = hh * HC
                        cw = min(HC, CPB1 - t0)
                        if cw <= 0:
                            break
                        xgt = xgp.tile([128, HC * F], f8, tag="xg")
                        nc.sync.dma_start(
                            out=xgt[:, 0:cw * F].rearrange("p (c k) -> p c k", k=F),
                            in_=xg[:, CPB1 * j + t0:CPB1 * j + t0 + cw, :])
                        oht = ohp.tile([128, HC * 128], f8, tag="oh")
                        c0 = (CPB1 * j + t0) * 128
                        nc.sync.dma_start(out=oht[:, 0:cw * 128],
                                          in_=oh1[:, c0:c0 + cw * 128])
                        for tl in range(cw):
                            t = t0 + tl
                            nc.tensor.matmul(
                                pre[:], lhsT=xgt[:, tl * F:(tl + 1) * F],
                                rhs=oht[:, tl * 128:(tl + 1) * 128],
                                start=(t == 0), stop=(t == CPB1 - 1))
                    preS = wp.tile([F, 128], bf16, tag="preS")
                    nc.vector.tensor_copy(preS[:], pre[:])
                    h1T = psB.tile([H, 128], f32, tag="h1T")
                    nc.tensor.matmul(h1T[:], lhsT=W1_t[:], rhs=preS[:],
                                     start=True, stop=True)
                    h1Ts = wp.tile([H, 128], f32, tag="h1Ts")
                    nc.vector.tensor_copy(h1Ts[:], h1T[:])
                    h1n = psB.tile([128, H], f32, tag="h1n")
                    nc.tensor.transpose(h1n[:], h1Ts[:], ident_t[:])
                    s = wp.tile([128, H], f32, tag="ep")
                    nc.vector.tensor_scalar_mul(s[:], h1n[:], dnv_t[:, j:j + 1])
                    nc.vector.tensor_add(s[:], s[:], b1_t[:])
                    nc.vector.tensor_scalar(
                        vcol_t[:, H * j:H * (j + 1)], s[:], 0.0,
                        dnv_t[:, j:j + 1], op0=OP.max, op1=OP.mult)
                nc.sync.dma_start(
                    out=vb.ap().rearrange("(j p) f -> p j f", p=128),
                    in_=vcol_t[:].rearrange("p (j f) -> p j f", f=H))
            nc.gpsimd.collective_compute("AllGather", OP.bypass, replica_groups=RG,
                                         ins=[vb.ap().opt()], outs=[vf.ap().opt()])

            # ---------------- layer 2: window-pair DoubleRow aggregation --
            with nc.named_scope("l2agg"):
                u_t = cp.tile([128, NW * H], f8)
                nc.sync.dma_start(
                    out=u_t[:].rearrange("p (w f) -> p w f", f=H),
                    in_=vf.ap().rearrange("(w p) f -> p w f", p=128))
                GS = [(0, 512), (512, 512), (1024, 256)]
                Sg = [psS.tile([H, w], f32, tag=f"S{gi}", name=f"S{gi}")
                      for gi, (_, w) in enumerate(GS)]
                for pt in range(NT2):
                    t2t = t2p.tile([128, PAIRS_PER_TILE * 2 * SH], f8, tag="t2")
                    nc.sync.dma_start(
                        out=t2t[:].rearrange("p (r q) -> p r q", q=2 * SH),
                        in_=T2[:, pt * PAIRS_PER_TILE:(pt + 1) * PAIRS_PER_TILE, :])
                    for pr in range(PAIRS_PER_TILE):
                        w = pt * PAIRS_PER_TILE + pr
                        lw = u_t[:, 2 * w * H:(2 * w + 2) * H].rearrange(
                            "p (two f) -> p two f", two=2)
                        for gi, (g0, gw) in enumerate(GS):
                            rh = t2t[:, pr * 2 * SH:(pr + 1) * 2 * SH].rearrange(
                                "p (two d) -> p two d", two=2)[:, :, g0:g0 + gw]
                            nc.tensor.matmul(
                                Sg[gi][:], lhsT=lw, rhs=rh,
                                start=(w == 0), stop=(w == NWP - 1),
                                perf_mode=mybir.MatmulPerfMode.DoubleRow)
                for gi, (g0, gw) in enumerate(GS):
                    Ss = wp.tile([H, 512], bf16, tag="Ss")
                    nc.vector.tensor_copy(Ss[:, 0:gw], Sg[gi][:])
                    h2g = psB.tile([H, 512], f32, tag="h2g")
                    nc.tensor.matmul(h2g[:, 0:gw], lhsT=W2_t[:], rhs=Ss[:, 0:gw],
                                     start=True, stop=True)
                    nc.vector.tensor_scalar(
                        h2T_t[:, g0:g0 + gw], h2g[:, 0:gw], b2_t[:, 0:1],
                        None, op0=OP.add)
                nc.sync.dma_start(out=hb.ap(), in_=h2T_t[:])
            nc.gpsimd.collective_compute("AllGather", OP.bypass, replica_groups=RG,
                                         ins=[hb.ap().opt()], outs=[h2f.ap().opt()])
            _agg.close()
            psE = _stk.enter_context(tc.tile_pool(name="psE", bufs=2, space="PSUM"))
            widep = _stk.enter_context(tc.tile_pool(name="widep", bufs=2))

            # ---------------- affinity + sigmoid + packed writes ----------
            with nc.named_scope("affprep"):
                af2_t = cp.tile([H, NPAD], bf16)
                nc.sync.dma_start(
                    out=af2_t[:].rearrange("f (s n) -> f s n", n=SH),
                    in_=h2f.ap().rearrange("(s f) n -> f s n", f=H))
                h2fl = h2f.ap().rearrange("p (b n) -> (p b) n", n=128)
                lhsTs = []
                for i in range(BPC):
                    hr = cp.tile([H, 128], bf16, tag=f"hr{i}")
                    nc.gpsimd.indirect_dma_start(
                        out=hr[:], out_offset=None, in_=h2fl,
                        in_offset=bass.IndirectOffsetOnAxis(
                            ap=rowi_t[:, i:i + 1], axis=0))
                    lhsTs.append(hr)

            with nc.named_scope("aff"):
                for i in range(BPC):
                    Wi = NPAD - 1024 * i
                    for a0 in range(0, Wi, AW):
                        aw = min(AW, Wi - a0)
                        wt = widep.tile([128, AW], bf16, tag="wide")
                        for k in range(0, aw, 2048):
                            kw = min(2048, aw - k)
                            pa = psE.tile([128, 2048], f32, tag="affps")
                            for q in range(0, kw, 512):
                                c0 = 1024 * i + a0 + k + q
                                nc.tensor.matmul(
                                    pa[:, q:q + 512], lhsT=lhsTs[i][:],
                                    rhs=af2_t[:, c0:c0 + 512],
                                    start=True, stop=True)
                            nc.scalar.activation(wt[:, k:k + kw],
                                                 pa[:, 0:kw], AF.Sigmoid)
                        nc.sync.dma_start(out=outs[i][:, a0:a0 + aw],
                                          in_=wt[:, 0:aw])

            if debug:
                d = nc.dram_tensor("dbg_vf", [NPAD, H], f8,
                                   kind="ExternalOutput")
                nc.sync.dma_start(out=d.ap(), in_=vf.ap())
                d = nc.dram_tensor("dbg_h2f", [128, SH], bf16,
                                   kind="ExternalOutput")
                nc.sync.dma_start(out=d.ap(), in_=h2f.ap())

    nc.compile()
    return nc


# ------------------------------------------------------------------ host ----

def preprocess(x, edge_index, W1, b1, W2, b2, cfg):
    """Build the 8 per-core input maps. Returns (in_maps, cpb_needed)."""
    CPB1, G1 = cfg["CPB1"], cfg["G1"]

    x = np.asarray(x, dtype=np.float32)
    src = np.asarray(edge_index[0], dtype=np.int64)
    dst = np.asarray(edge_index[1], dtype=np.int64)
    W1 = np.asarray(W1, np.float32).astype(BF)
    W2 = np.asarray(W2, np.float32).astype(BF)
    b1 = np.asarray(b1, np.float32).reshape(1, H)
    b2 = np.asarray(b2, np.float32).reshape(H, 1)

    xp = np.zeros((NPAD, F), np.float32)
    xp[:N] = x
    deg = (np.bincount(dst, minlength=NPAD) + 1).astype(np.float64)
    dinv = (1.0 / np.sqrt(deg)).astype(np.float32)
    xs8 = (xp * dinv[:, None]).astype(F8)          # x * dinv[src], fp8

    loop = np.arange(NPAD, dtype=np.int64)
    s_all = np.concatenate([src, loop])
    d_all = np.concatenate([dst, loop])
    order = np.argsort(d_all, kind="stable")
    s_s = s_all[order].astype(np.int64)
    d_s = d_all[order].astype(np.int64)

    ident = np.eye(H, dtype=np.float32)
    b1b = np.broadcast_to(b1, (128, H)).copy()
    one8 = np.float32(1.0).astype(F8)

    in_maps = []
    cpb_needed = 0
    for c in range(NCORES):
        lo, hi = SH * c, SH * (c + 1)
        a, b = np.searchsorted(d_s, [lo, hi])
        s_c, d_c = s_s[a:b], d_s[a:b]
        blk = (d_c - lo) // 128
        bounds = np.searchsorted(blk, np.arange(BPC + 1))

        # layer-1 slots: per dst block, edges packed into [128, CPB1] slots
        slot_src = np.zeros((128, G1), np.int64)
        oh_flat = np.zeros((128, G1 * 128), np.uint8)
        for j in range(BPC):
            sl = slice(bounds[j], bounds[j + 1])
            sj, dj = s_c[sl], d_c[sl]
            m = len(sj)
            cpb_needed = max(cpb_needed, -(-m // 128))
            if m > CPB1 * 128:
                return None, cpb_needed
            e = np.arange(m)
            t = CPB1 * j + e // 128
            p = e % 128
            slot_src[p, t] = sj
            dloc = dj - lo - 128 * j
            oh_flat[p, t * 128 + dloc] = one8.view(np.uint8)
        xgc = xs8[slot_src]                         # [128, G1, F] fp8
        # zero out the unused tail slots (slot_src defaulted to node 0)
        for j in range(BPC):
            m = bounds[j + 1] - bounds[j]
            t_full = m // 128
            if t_full < CPB1:
                p0 = m % 128
                xgc[p0:, CPB1 * j + t_full] = 0
                if t_full + 1 < CPB1:
                    xgc[:, CPB1 * j + t_full + 1:CPB1 * (j + 1)] = 0

        # layer-2 T matrix: [p=s%128, w=s//128, dloc] = mult * dinv[dst]
        T2f = np.zeros((128, NW, SH), np.float32)
        np.add.at(T2f, (s_c % 128, s_c // 128, d_c - lo), dinv[d_c])
        T2c = T2f.astype(F8).reshape(128, NWP, 2 * SH)

        # aff lhsT row indices into flat (p, b) view of h2f [128, SH]
        ii = np.arange(BPC)
        k = 8 * ii + c
        sc, bc = k // BPC, k % BPC
        q = np.arange(H)
        rowi = ((H * sc[None, :] + q[:, None]) * BPC + bc[None, :]).astype(np.int32)

        in_maps.append({
            "W1": W1, "W2": W2, "b1": b1b, "b2": b2,
            "dnv": np.ascontiguousarray(
                dinv[lo + 128 * np.arange(BPC)[None, :] + np.arange(128)[:, None]]),
            "xg": xgc.view(F8),
            "oh1": oh_flat.view(F8),
            "T2": T2c,
            "ident": ident, "rowi": rowi,
        })
    return in_maps, cpb_needed


def assemble(results, cfg):
    T = N * (N - 1) // 2
    row_off = np.zeros(N + 1, np.int64)
    np.cumsum((N - 1) - np.arange(N), out=row_off[1:])
    out = np.empty(T, np.float32)
    for c in range(NCORES):
        for i in range(BPC):
            reg = np.asarray(results[c][f"out{i}"]).astype(np.float32)
            r0 = 128 * (8 * i + c)
            if r0 >= N - 1:
                continue
            base = 1024 * i
            for p in range(min(128, N - 1 - r0)):
                r = r0 + p
                L = N - 1 - r
                cs = r + 1 - base
                out[row_off[r]:row_off[r] + L] = reg[p, cs:cs + L]
    return out.reshape(-1, 1)


_NC_CACHE = {}


def _get_nc(cfg, debug=False):
    key = (cfg["CPB1"], debug)
    if key not in _NC_CACHE:
        _NC_CACHE[key] = build_nc(cfg, debug=debug)
    return _NC_CACHE[key]


def run(inputs, cfg, trace=False, trace_kwargs=None, debug=False):
    """Run the kernel for the given cfg; returns (BassKernelResults, cfg)."""
    from concourse.bass_utils import run_bass_kernel_spmd

    in_maps, cpb_needed = preprocess(
        inputs["x"], inputs["edge_index"], inputs["W1"], inputs["b1"],
        inputs["W2"], inputs["b2"], cfg)
    if in_maps is None:
        cfg = _cfg(CPB1=cpb_needed)
        in_maps, _ = preprocess(
            inputs["x"], inputs["edge_index"], inputs["W1"], inputs["b1"],
            inputs["W2"], inputs["b2"], cfg)
    nc = _get_nc(cfg, debug=debug)
    res = run_bass_kernel_spmd(nc, in_maps, core_ids=list(range(NCORES)),
                               trace=trace, **(trace_kwargs or {}))
    return res, cfg


def kernel(**inputs) -> np.ndarray:
    res, cfg = run(inputs, FULL, trace=False)
    return assemble(res.results, cfg)


if __name__ == "__main__":
    pass
